# revision 1
# baseline (speedup 1.0000x reference)
"""AELN-GCN (edge-weighted ChebConv K=3, 4 layers) on 8 TRN2 NeuronCores.

Strategy (graph/data parallel, dst-sharded edges):
  - nodes sharded 8 ways by range; core c owns dst nodes [c*NSH, (c+1)*NSH)
  - edge_net runs on-device twice: once over dst-sorted edges (per-edge
    weights for propagation), once over src-sorted edges (degree).
  - degree/dinv are computed per src-shard and stay local (each core only
    ever scales its own node rows).
  - per Chebyshev propagate: each core builds a node-major bf16 table of
    dinv*x for its shard, AllGathers the full table, then per dst-tile
    (128 dsts) gathers all source rows with one indirect DMA (int32 row
    indices) and aggregates with per-128-edge-chunk one-hot matmuls into
    PSUM.  y = -dinv[dst] * sum(ew * dinv[src] * x[src]).
  - dense 128x128 weights replicated; cheb matmuls run feature-major.

Host side does only index bookkeeping: sharding, sorting edges by
dst/src tile, padding tiles to a fixed chunk count, and permuting the
edgenet input rows to match the on-device processing order.
"""

import sys

sys.path.insert(0, "/opt/trn_rl_repo")

import numpy as np
import ml_dtypes

P = 128  # partitions
EF = 64  # edge feature dim
HF = 32  # edgenet hidden dim
F = 128  # node feature dim
NGL = 4

BF16 = ml_dtypes.bfloat16


# ----------------------------------------------------------------------------
# config
# ----------------------------------------------------------------------------
class Cfg:
    def __init__(self, n_nodes, n_edges, n_cores, ct=None, ct2=None):
        assert n_nodes % n_cores == 0
        self.N = n_nodes
        self.E = n_edges
        self.NC = n_cores
        self.NSH = n_nodes // n_cores  # nodes per shard
        self.DT = -(-self.NSH // P)  # dst tiles per core
        self.DTP = self.DT * P  # padded shard rows
        self.TROWS = self.NC * self.DTP  # all-gathered table rows
        self.CT = ct  # chunks per dst tile (set by prep)
        self.CT2 = ct2  # chunks per src tile (set by prep)
        # gather bucketing (dma_gather uses int16 indices, <=32768 rows)
        self.NB = -(-self.TROWS // 32768)
        assert self.TROWS % self.NB == 0
        self.BSZ = self.TROWS // self.NB
        # tiles per gather group: largest divisor of DT <= 7
        self.TG = max(d for d in range(1, 8) if self.DT % d == 0)
        self.NG = self.DT // self.TG
        self.CTB = None  # chunks per (tile, bucket) span (set by prep)

    @property
    def GN(self):
        # indices per dma_gather call (one (tile, bucket) span; HW caps
        # dma_gather around 1024-2047 indices per call)
        return self.CTB * P

    @property
    def EP(self):
        return self.DT * self.CT * P

    @property
    def EP2(self):
        return self.DT * self.CT2 * P


# ----------------------------------------------------------------------------
# host prep
# ----------------------------------------------------------------------------
def _shard_order(node_of_edge, cfg):
    """Per-core [DT, P, CT]-layout slot assignment for edges grouped by
    `node_of_edge` (dst for prop shards, src for deg shards).

    Returns per-core dicts with:
      eid   [DT, P, CT] int64  original edge id (-1 for pad slots)
      rel   [DT, P, CT] int32  node - tile_base (-1 for pad slots)
    and the required max chunk count.
    """
    cores = []
    max_ct = 1
    order = np.argsort(node_of_edge, kind="stable")
    node_sorted = node_of_edge[order]
    # boundaries per core
    for c in range(cfg.NC):
        lo = np.searchsorted(node_sorted, c * cfg.NSH)
        hi = np.searchsorted(node_sorted, (c + 1) * cfg.NSH)
        eids = order[lo:hi]
        locs = node_sorted[lo:hi] - c * cfg.NSH
        tiles = locs // P
        counts = np.bincount(tiles, minlength=cfg.DT)
        max_ct = max(max_ct, int(-(-counts.max() // P)))
        cores.append((eids, locs, tiles, counts))
    return cores, max_ct


def _fill_slots(cores, cfg, ct):
    out = []
    for eids, locs, tiles, counts in cores:
        eid = np.full((cfg.DT, ct * P), -1, dtype=np.int64)
        rel = np.full((cfg.DT, ct * P), -1, dtype=np.int32)
        starts = np.zeros(cfg.DT + 1, dtype=np.int64)
        np.cumsum(counts, out=starts[1:])
        # edges are sorted by node => per tile contiguous
        for t in range(cfg.DT):
            n_t = counts[t]
            if n_t == 0:
                continue
            sl = slice(starts[t], starts[t + 1])
            eid[t, :n_t] = eids[sl]
            rel[t, :n_t] = locs[sl] - t * P
        # logical order within tile is q = j*P + p -> reshape [CT, P] then
        # transpose to storage [P, CT]
        eid = eid.reshape(cfg.DT, ct, P).transpose(0, 2, 1).copy()
        rel = rel.reshape(cfg.DT, ct, P).transpose(0, 2, 1).copy()
        out.append((eid, rel))
    return out


def _fill_slots_bucketed(cores, cfg, trow_of_src):
    """Dst-shard slot assignment with per-(tile, bucket-of-src) spans of
    fixed CTB chunks.  Logical slot (t, j=b*CTB+cb, p); q = j*P + p.

    Returns per-core (eid [DT,P,CT], rel [DT,P,CT], lidx [DT,CT*P] logical
    int16 local table row) and sets cfg.CTB / cfg.CT.
    """
    # first pass: per (core, tile, bucket) counts to size CTB
    percore = []
    max_ctb = 1
    for eids, locs, tiles, counts in cores:
        tr = trow_of_src[eids]
        b = tr // cfg.BSZ
        key = (tiles * cfg.NB + b).astype(np.int64)
        order = np.argsort(key, kind="stable")
        eids_s, locs_s, key_s = eids[order], locs[order], key[order]
        cnt = np.bincount(key_s, minlength=cfg.DT * cfg.NB)
        max_ctb = max(max_ctb, int(-(-cnt.max() // P)))
        percore.append((eids_s, locs_s, key_s, cnt, tr[order]))
    cfg.CTB = max(cfg.CTB or 0, max_ctb)
    cfg.CT = cfg.NB * cfg.CTB
    while (cfg.DT * cfg.CT) % 4 != 0:
        cfg.CTB += 1
        cfg.CT = cfg.NB * cfg.CTB
    out = []
    span = cfg.CTB * P
    for eids_s, locs_s, key_s, cnt, tr_s in percore:
        eid = np.full((cfg.DT * cfg.NB, span), -1, dtype=np.int64)
        rel = np.full((cfg.DT * cfg.NB, span), -1, dtype=np.int32)
        lidx = np.zeros((cfg.DT * cfg.NB, span), dtype=np.int32)
        starts = np.zeros(cfg.DT * cfg.NB + 1, dtype=np.int64)
        np.cumsum(cnt, out=starts[1:])
        for k in range(cfg.DT * cfg.NB):
            n_k = cnt[k]
            if n_k == 0:
                continue
            sl = slice(starts[k], starts[k + 1])
            eid[k, :n_k] = eids_s[sl]
            t = k // cfg.NB
            rel[k, :n_k] = locs_s[sl] - t * P
            lidx[k, :n_k] = tr_s[sl] - (k % cfg.NB) * cfg.BSZ
        # [DT, NB, CTB*P] -> logical [DT, CT*P] with j = b*CTB+cb
        eid = eid.reshape(cfg.DT, cfg.CT * P)
        rel = rel.reshape(cfg.DT, cfg.CT * P)
        lidx = lidx.reshape(cfg.DT, cfg.CT * P)
        assert lidx.max() < 32768 and lidx.min() >= 0
        out.append((eid, rel, lidx.astype(np.int16)))
    return out


def host_prep(inputs, cfg):
    """Build per-core input maps."""
    feats = np.asarray(inputs["features"], dtype=np.float32)
    egin = np.asarray(inputs["edgenet_input"], dtype=np.float32)
    E1 = np.asarray(inputs["E1"], dtype=np.float32)
    e1b = np.asarray(inputs["e1b"], dtype=np.float32)
    E2 = np.asarray(inputs["E2"], dtype=np.float32)
    e2b = np.asarray(inputs["e2b"], dtype=np.float32)
    W = np.asarray(inputs["W"], dtype=np.float32)
    b = np.asarray(inputs["b"], dtype=np.float32)
    ei = np.asarray(inputs["edge_index"])
    src = ei[0].astype(np.int64)
    dst = ei[1].astype(np.int64)

    # table row of global node n
    def trow(n):
        return (n // cfg.NSH) * cfg.DTP + (n % cfg.NSH)

    trow_src = trow(src).astype(np.int64)

    dcores, _ = _shard_order(dst, cfg)
    scores, ct_s = _shard_order(src, cfg)
    cfg.CT2 = max(cfg.CT2 or 0, ct_s)
    while (cfg.DT * cfg.CT2) % 4 != 0:
        cfg.CT2 += 1
    dslots = _fill_slots_bucketed(dcores, cfg, trow_src)
    sslots = _fill_slots(scores, cfg, cfg.CT2)

    # edgenet input with a zero pad row at index E
    eg_ext = np.concatenate([egin, np.zeros((1, EF), np.float32)], axis=0)

    in_maps = []
    meta = []
    span = cfg.CTB * P
    for c in range(cfg.NC):
        eid_d, rel_d, lidx_d = dslots[c]  # [DT, CT*P] logical
        eid_s, rel_s = sslots[c]

        # storage layout [DT, P, CT] for per-edge streams
        def to_storage(a):
            return np.ascontiguousarray(
                a.reshape(cfg.DT, cfg.CT, P).transpose(0, 2, 1)
            )

        rel_d_st = to_storage(rel_d)
        eid_d_st = to_storage(eid_d)

        # gather index arrays: [DT, NB, 128, GN//16] int16, wrapped by 16
        lx = lidx_d.reshape(cfg.DT, cfg.NB, span)
        gidx = lx.reshape(cfg.DT, cfg.NB, cfg.GN // 16, 16).transpose(0, 1, 3, 2)
        gidx = np.ascontiguousarray(np.tile(gidx, (1, 1, 8, 1)))  # 128 partitions

        featT = np.zeros((F, cfg.DTP), np.float32)
        sh = feats[c * cfg.NSH : (c + 1) * cfg.NSH]
        featT[:, : cfg.NSH] = sh.T

        m = {
            "featT": featT,
            "eg": np.ascontiguousarray(eg_ext[eid_d_st.reshape(-1)].T),  # [64, EP]
            "eg2": np.ascontiguousarray(eg_ext[eid_s.reshape(-1)].T),  # [64, EP2]
            "gidx": gidx,  # [NG,NB,16,GN/16] i16
            "dstrel": rel_d_st.astype(BF16),  # [DT,P,CT] bf16
            "srcrel2": rel_s.astype(np.float32),  # [DT,P,CT2] f32
            "E1": E1,
            "e1b": e1b.reshape(HF, 1),
            "E2": E2,
            "e2b": e2b.reshape(1, 1),
            "W": W,
            "bT": np.ascontiguousarray(b.T),  # [128, NGL]
        }
        in_maps.append(m)
        # emulation metadata: full table row per storage slot
        srcidx_full = np.where(
            eid_d >= 0,
            lidx_d.astype(np.int64)
            + (np.arange(cfg.CT * P) // span)[None, :] * cfg.BSZ,
            0,
        )
        meta.append({"srcidx": to_storage(srcidx_full).astype(np.int32)})
    return in_maps, meta


# ----------------------------------------------------------------------------
# numpy emulation of the device program (structure check; f32 math)
# ----------------------------------------------------------------------------
def emulate(in_maps, cfg, meta):
    NC = cfg.NC

    def edgenet(eg):
        h = np.maximum(eg.T @ in_maps[0]["E1"] + in_maps[0]["e1b"][:, 0], 0.0)
        z = h @ in_maps[0]["E2"] + in_maps[0]["e2b"][0]
        return 1.0 / (1.0 + np.exp(-z[:, 0]))

    ew1 = [edgenet(m["eg"]) for m in in_maps]  # [EP] in storage order
    ew2 = [edgenet(m["eg2"]) for m in in_maps]

    # deg per src shard
    dinv = []
    for c in range(NC):
        rel = in_maps[c]["srcrel2"].astype(np.int64)  # [DT,P,CT2]
        deg = np.zeros(cfg.DTP)
        w = ew2[c].reshape(cfg.DT, P, cfg.CT2)
        valid = rel >= 0
        tt, pp, jj = np.nonzero(valid)
        np.add.at(deg, tt * P + rel[tt, pp, jj], w[tt, pp, jj])
        dv = np.where(deg > 0, 1.0 / np.sqrt(np.maximum(deg, 1e-12)), 0.0)
        dinv.append(dv)

    x_temp = [m["featT"].astype(np.float32).copy() for m in in_maps]  # [128, DTP]

    def build_table(arrs):
        # per-core node-major dinv*x, concatenated
        tab = np.zeros((cfg.TROWS, F), np.float32)
        for c in range(NC):
            tab[c * cfg.DTP : (c + 1) * cfg.DTP] = (arrs[c] * dinv[c][None, :]).T
        return tab

    def prop(tab):
        # returns per-core [128, DTP] = -dinv * sum(ew * tab[src])
        outs = []
        for c in range(NC):
            rel = in_maps[c]["dstrel"].astype(np.float32)  # [DT,P,CT] bf16->f32
            sidx = meta[c]["srcidx"].astype(np.int64)
            w = ew1[c].reshape(cfg.DT, P, cfg.CT)
            y = np.zeros((F, cfg.DTP), np.float32)
            g = tab[sidx]  # [DT,P,CT,128]
            wm = np.where(rel >= 0, w, 0.0)
            contrib = g * wm[..., None]  # [DT,P,CT,128]
            for t in range(cfg.DT):
                reli = rel[t].astype(np.int64)  # [P, CT]
                acc = np.zeros((P, F), np.float32)
                valid = reli >= 0
                pp, jj = np.nonzero(valid)
                np.add.at(acc, reli[pp, jj], contrib[t, pp, jj])
                y[:, t * P : (t + 1) * P] = acc.T
            outs.append(-dinv[c][None, :] * y)
        return outs

    W = in_maps[0]["W"]
    bT = in_maps[0]["bT"]
    for l in range(NGL):
        tabA = build_table(x_temp)
        t1 = prop(tabA)
        tabB = build_table(t1)
        p2 = prop(tabB)
        outs = []
        for c in range(NC):
            t2 = 2.0 * p2[c] - x_temp[c]
            o = (
                W[l, 0].T @ x_temp[c]
                + W[l, 1].T @ t1[c]
                + W[l, 2].T @ t2
                + bT[:, l : l + 1]
            )
            outs.append(np.maximum(o, 0.0))
        if l == 0:
            x_temp = outs
        elif l < NGL - 1:
            x_temp = [x_temp[c] + outs[c] for c in range(NC)]
        else:
            x_temp = outs

    out = np.zeros((cfg.N, F), np.float32)
    for c in range(NC):
        out[c * cfg.NSH : (c + 1) * cfg.NSH] = x_temp[c][:, : cfg.NSH].T
    return out


# ----------------------------------------------------------------------------
# assemble final output from per-core results
# ----------------------------------------------------------------------------
def assemble(results, cfg):
    out = np.zeros((cfg.N, F), np.float32)
    for c in range(cfg.NC):
        outT = np.asarray(results[c]["outT"], dtype=np.float32)
        out[c * cfg.NSH : (c + 1) * cfg.NSH] = outT[:, : cfg.NSH].T
    return out


# ----------------------------------------------------------------------------
# bass kernel builder
# ----------------------------------------------------------------------------
def build_nc(cfg, do_compile=True, debug_outputs=False):
    import concourse.bass as bass
    import concourse.bacc as bacc
    import concourse.mybir as mybir
    import concourse.tile as tile
    from concourse.masks import make_identity

    dt = mybir.dt
    AF = mybir.ActivationFunctionType
    ALU = mybir.AluOpType

    NC, DT, CT, CT2, DTP, TROWS = cfg.NC, cfg.DT, cfg.CT, cfg.CT2, cfg.DTP, cfg.TROWS
    NB, BSZ, TG, NG, CTB, GN = cfg.NB, cfg.BSZ, cfg.TG, cfg.NG, cfg.CTB, cfg.GN
    EP, EP2 = cfg.EP, cfg.EP2
    core_ids = list(range(NC))

    nc = bacc.Bacc()

    # ---- external I/O ----
    featT_d = nc.declare_dram_parameter("featT", [F, DTP], dt.float32, isOutput=False)
    eg_d = nc.declare_dram_parameter("eg", [EF, EP], dt.float32, isOutput=False)
    eg2_d = nc.declare_dram_parameter("eg2", [EF, EP2], dt.float32, isOutput=False)
    gidx_d = nc.declare_dram_parameter(
        "gidx", [DT, cfg.NB, P, cfg.GN // 16], dt.int16, isOutput=False
    )
    dstrel_d = nc.declare_dram_parameter(
        "dstrel", [DT, P, CT], dt.bfloat16, isOutput=False
    )
    srcrel2_d = nc.declare_dram_parameter(
        "srcrel2", [DT, P, CT2], dt.float32, isOutput=False
    )
    E1_d = nc.declare_dram_parameter("E1", [EF, HF], dt.float32, isOutput=False)
    e1b_d = nc.declare_dram_parameter("e1b", [HF, 1], dt.float32, isOutput=False)
    E2_d = nc.declare_dram_parameter("E2", [HF, 1], dt.float32, isOutput=False)
    e2b_d = nc.declare_dram_parameter("e2b", [1, 1], dt.float32, isOutput=False)
    W_d = nc.declare_dram_parameter("W", [NGL, 3, F, F], dt.float32, isOutput=False)
    bT_d = nc.declare_dram_parameter("bT", [F, NGL], dt.float32, isOutput=False)
    outT_d = nc.declare_dram_parameter("outT", [F, DTP], dt.float32, isOutput=True)
    if debug_outputs:
        dbg_ew1 = nc.declare_dram_parameter("dbg_ew1", [EP], dt.float32, isOutput=True)
        dbg_dinv = nc.declare_dram_parameter("dbg_dinv", [DTP], dt.float32, isOutput=True)
        dbg_xfull = nc.declare_dram_parameter(
            "dbg_xfull", [TROWS, F], dt.bfloat16, isOutput=True
        )
        dbg_t1 = nc.declare_dram_parameter("dbg_t1", [F, DTP], dt.bfloat16, isOutput=True)
        dbg_xg = nc.declare_dram_parameter(
            "dbg_xg", [P, CT * F], dt.bfloat16, isOutput=True
        )
        dbg_drep = nc.declare_dram_parameter(
            "dbg_drep", [F, DTP], dt.bfloat16, isOutput=True
        )

    # ---- internal dram ----
    ew1_d = nc.dram_tensor("ew1_buf", [EP], dt.float32)
    ew2_d = nc.dram_tensor("ew2_buf", [EP2], dt.float32)
    dinv_d = nc.dram_tensor("dinv_buf", [DTP], dt.float32)
    xsrc_a = nc.dram_tensor("xsrc_a", [DTP, F], dt.bfloat16)
    xsrc_b = nc.dram_tensor("xsrc_b", [DTP, F], dt.bfloat16)
    xfull_a = nc.dram_tensor("xfull_a", [TROWS, F], dt.bfloat16, addr_space="Shared")
    xfull_b = nc.dram_tensor("xfull_b", [TROWS, F], dt.bfloat16, addr_space="Shared")

    EB = 512  # edgenet block columns
    n_eb1 = EP // EB if EP % EB == 0 else None
    assert EP % EB == 0 and EP2 % EB == 0, (EP, EP2)

    with tile.TileContext(nc, num_cores=NC) as tc:
        with (
            tc.tile_pool(name="res", bufs=1) as res,
            tc.tile_pool(name="resw", bufs=1) as resw,
        ):
            # ---- resident tiles ----
            x_temp = res.tile([F, DTP], dt.float32, tag="x_temp")
            t1 = res.tile([F, DTP], dt.bfloat16, tag="t1")
            drep1 = res.tile([F, DTP], dt.bfloat16, tag="drep1")  # -dinv
            s2stage = res.tile([F, DTP], dt.bfloat16, tag="s2stage")  # prop(t1)
            dinv_sb = res.tile([P, DT], dt.float32, tag="dinv_sb")
            deg_sb = res.tile([P, DT], dt.float32, tag="deg_sb")
            ident_f = res.tile([P, P], dt.float32, tag="ident_f")
            ident_b = res.tile([P, P], dt.bfloat16, tag="ident_b")
            iota_i = res.tile([P, P], dt.int32, tag="iota_i")
            iota_b = res.tile([P, P], dt.bfloat16, tag="iota_b")
            iota_f = res.tile([P, P], dt.float32, tag="iota_f")
            ones_sb = res.tile([1, P], dt.float32, tag="ones_sb")
            E1_sb = resw.tile([EF, HF], dt.bfloat16, tag="E1_sb")
            e1b_sb = resw.tile([HF, 1], dt.float32, tag="e1b_sb")
            E2_sb = resw.tile([HF, 1], dt.bfloat16, tag="E2_sb")
            e2b_sb = resw.tile([1, 1], dt.float32, tag="e2b_sb")
            bT_sb = resw.tile([F, NGL], dt.float32, tag="bT_sb")
            W0_sb = resw.tile([F, NGL * F], dt.bfloat16, tag="W0_sb")
            Wb_sb = resw.tile([F, NGL * 2 * F], dt.bfloat16, tag="Wb_sb")

            make_identity(nc, ident_f[:])
            nc.vector.tensor_copy(ident_b[:], ident_f[:])
            nc.gpsimd.iota(iota_i[:], pattern=[[1, P]], base=0, channel_multiplier=0)
            nc.vector.tensor_copy(iota_b[:], iota_i[:])
            nc.vector.tensor_copy(iota_f[:], iota_i[:])
            nc.gpsimd.memset(ones_sb[:], 1.0)
            nc.gpsimd.dma_start(out=E1_sb[:], in_=E1_d[:, :])
            nc.sync.dma_start(out=e1b_sb[:], in_=e1b_d[:, :])
            nc.gpsimd.dma_start(out=E2_sb[:], in_=E2_d[:, :])
            nc.sync.dma_start(out=e2b_sb[:], in_=e2b_d[:, :])
            nc.sync.dma_start(out=bT_sb[:], in_=bT_d[:, :])
            for l in range(NGL):
                nc.gpsimd.dma_start(
                    out=W0_sb[:, l * F : (l + 1) * F], in_=W_d[l, 0, :, :]
                )
                for k in (1, 2):
                    nc.gpsimd.dma_start(
                        out=Wb_sb[:, (l * 2 + k - 1) * F : (l * 2 + k) * F],
                        in_=W_d[l, k, :, :],
                    )
            nc.sync.dma_start(out=x_temp[:], in_=featT_d[:, :])

            # ================= phase 1: edgenet (both orders) + deg =========
            with (
                tc.tile_pool(name="egp", bufs=3) as egp,
                tc.tile_pool(name="hp", bufs=2) as hp,
                tc.tile_pool(name="hpp", bufs=2, space="PSUM") as hpp,
                tc.tile_pool(name="ewp", bufs=2) as ewp,
                tc.tile_pool(name="ewpp", bufs=2, space="PSUM") as ewpp,
                tc.tile_pool(name="degp", bufs=3) as degp,
                tc.tile_pool(name="degpp", bufs=2, space="PSUM") as degpp,
            ):

                def edgenet(eg_dram, ew_dram, n_cols):
                    for blk in range(n_cols // EB):
                        egt = egp.tile([EF, EB], dt.bfloat16, tag="egt")
                        nc.gpsimd.dma_start(
                            out=egt[:], in_=eg_dram[:, blk * EB : (blk + 1) * EB]
                        )
                        ph = hpp.tile([HF, EB], dt.float32, tag="ph", space="PSUM")
                        nc.tensor.matmul(
                            out=ph[:],
                            lhsT=E1_sb[:],
                            rhs=egt[:],
                            start=True,
                            stop=True,
                        )
                        hs = hp.tile([HF, EB], dt.bfloat16, tag="hs")
                        nc.scalar.activation(hs[:], ph[:], AF.Relu, bias=e1b_sb[:, :1])
                        pw = ewpp.tile([1, EB], dt.float32, tag="pw", space="PSUM")
                        nc.tensor.matmul(
                            out=pw[:],
                            lhsT=E2_sb[:],
                            rhs=hs[:],
                            start=True,
                            stop=True,
                        )
                        ews = ewp.tile([1, EB], dt.float32, tag="ews")
                        nc.scalar.activation(
                            ews[:], pw[:], AF.Sigmoid, bias=e2b_sb[:1, :1]
                        )
                        nc.sync.dma_start(
                            out=ew_dram[None, blk * EB : (blk + 1) * EB],
                            in_=ews[:1, :],
                        )

                edgenet(eg_d, ew1_d, EP)
                edgenet(eg2_d, ew2_d, EP2)
                tc.strict_bb_all_engine_barrier()

                # deg: per src tile, one-hot matmul accumulate
                for t in range(DT):
                    relt = degp.tile([P, CT2], dt.float32, tag="relt")
                    nc.sync.dma_start(out=relt[:], in_=srcrel2_d[t, :, :])
                    ewt = degp.tile([P, CT2], dt.float32, tag="ewt2")
                    nc.sync.dma_start(
                        out=ewt[:],
                        in_=ew2_d[t * P * CT2 : (t + 1) * P * CT2].rearrange(
                            "(p j) -> p j", p=P
                        ),
                    )
                    pd = degpp.tile([P, 1], dt.float32, tag="pd", space="PSUM")
                    for j in range(CT2):
                        A2 = degp.tile([P, P], dt.float32, tag="A2")
                        nc.vector.tensor_tensor(
                            out=A2[:],
                            in0=relt[:, j : j + 1].to_broadcast([P, P]),
                            in1=iota_f[:],
                            op=ALU.is_equal,
                        )
                        nc.tensor.matmul(
                            out=pd[:],
                            lhsT=A2[:],
                            rhs=ewt[:, j : j + 1],
                            start=(j == 0),
                            stop=(j == CT2 - 1),
                        )
                    nc.vector.tensor_copy(deg_sb[:, t : t + 1], pd[:])

                tc.strict_bb_all_engine_barrier()
                # dinv = (deg>0) / sqrt(max(deg,1e-12))
                mx = degp.tile([P, DT], dt.float32, tag="mx")
                nc.vector.tensor_scalar_max(mx[:], deg_sb[:], 1e-12)
                rc = degp.tile([P, DT], dt.float32, tag="rc")
                nc.vector.reciprocal(rc[:], mx[:])
                sq = degp.tile([P, DT], dt.float32, tag="sq")
                nc.scalar.activation(sq[:], rc[:], AF.Sqrt)
                msk = degp.tile([P, DT], dt.float32, tag="msk")
                nc.vector.tensor_scalar(
                    out=msk[:], in0=deg_sb[:], scalar1=0.0, scalar2=None, op0=ALU.is_gt
                )
                nc.vector.tensor_tensor(
                    out=dinv_sb[:], in0=sq[:], in1=msk[:], op=ALU.mult
                )

                # flatten dinv to dram [DTP] (node-major order)
                pdv = degpp.tile([P, P], dt.float32, tag="pdv", space="PSUM", bufs=1)
                nc.tensor.transpose(out=pdv[:DT, :], in_=dinv_sb[:], identity=ident_f[:])
                dvt = degp.tile([DT, P], dt.float32, tag="dvt")
                nc.vector.tensor_copy(dvt[:], pdv[:DT, :])
                nc.sync.dma_start(
                    out=dinv_d[:].rearrange("(t p) -> t p", p=P), in_=dvt[:]
                )
                # replicate -dinv, -2dinv across partitions
                r0 = 0
                while r0 < DTP:
                    rb = min(EB, DTP - r0)
                    drow = degp.tile([1, EB], dt.float32, tag="drow", bufs=2)
                    nc.sync.dma_start(out=drow[:, :rb], in_=dinv_d[None, r0 : r0 + rb])
                    prr = degpp.tile([P, EB], dt.float32, tag="prr", space="PSUM", bufs=1)
                    nc.tensor.matmul(
                        out=prr[:, :rb],
                        lhsT=ones_sb[:],
                        rhs=drow[:, :rb],
                        start=True,
                        stop=True,
                    )
                    nc.scalar.activation(
                        drep1[:, r0 : r0 + rb], prr[:, :rb], AF.Copy, scale=-1.0
                    )
                    r0 += rb

            # ================= phase 2: layers ==============================
            def build_table(src_sb, src_dtype_f32, xsrc_dram, xfull_dram, pools):
                ttp, tpp = pools
                tc.strict_bb_all_engine_barrier()
                ident = ident_f if src_dtype_f32 else ident_b
                pdt = dt.float32 if src_dtype_f32 else dt.bfloat16
                for t in range(DT):
                    pt = tpp.tile([P, P], pdt, tag="pt", space="PSUM")
                    nc.tensor.transpose(
                        out=pt[:],
                        in_=src_sb[:, t * P : (t + 1) * P],
                        identity=ident[:],
                    )
                    xs = ttp.tile([P, P], dt.bfloat16, tag="xs")
                    nc.scalar.activation(
                        xs[:], pt[:], AF.Copy, scale=dinv_sb[:, t : t + 1]
                    )
                    nc.sync.dma_start(
                        out=xsrc_dram[t * P : (t + 1) * P, :], in_=xs[:]
                    )
                nc.gpsimd.collective_compute(
                    "AllGather",
                    mybir.AluOpType.bypass,
                    replica_groups=[core_ids],
                    ins=[xsrc_dram[:, :]],
                    outs=[xfull_dram[:, :]],
                )

            def prop_pass(xfull_dram, consume, pools):
                """Full propagate pass over all dst tiles.

                Gathers source rows group-by-group via dma_gather (int16
                bucketed indices), aggregates each tile into PSUM via one-hot
                matmuls, then calls consume(t, py) with the raw [F, 128d]
                PSUM aggregate."""
                gp, ap, ypp = pools
                for t in range(DT):
                    xg_bufs = []
                    for bkt in range(NB):
                        gi = gp.tile([P, GN // 16], dt.int16, tag=f"gi{bkt}", bufs=2)
                        nc.sync.dma_start(out=gi[:], in_=gidx_d[t, bkt, :, :])
                        xgb = gp.tile(
                            [P, CTB, F], dt.bfloat16, tag=f"xg{bkt}", bufs=2
                        )
                        # HW caps dma_gather somewhere in (1024, 2048) idxs
                        for s0 in range(0, CTB, 8):
                            s1 = min(s0 + 8, CTB)
                            n_s = (s1 - s0) * P
                            nc.gpsimd.dma_gather(
                                out_ap=xgb[:, s0:s1, :],
                                in_ap=xfull_dram[bkt * BSZ : (bkt + 1) * BSZ, :],
                                idxs_ap=gi[:, s0 * 8 : s1 * 8],
                                num_idxs=n_s,
                                num_idxs_reg=n_s,
                                elem_size=F,
                            )
                        xg_bufs.append(xgb)
                    drl = gp.tile([P, CT], dt.bfloat16, tag="drl")
                    nc.sync.dma_start(out=drl[:], in_=dstrel_d[t, :, :])
                    ewt = gp.tile([P, CT], dt.float32, tag="ewt1")
                    nc.sync.dma_start(
                        out=ewt[:],
                        in_=ew1_d[t * P * CT : (t + 1) * P * CT].rearrange(
                            "(p j) -> p j", p=P
                        ),
                    )
                    py = ypp.tile([P, P], dt.float32, tag="py", space="PSUM")
                    for j in range(CT):
                        bkt, cb = divmod(j, CTB)
                        A = ap.tile([P, P], dt.bfloat16, tag="A")
                        nc.vector.tensor_tensor(
                            out=A[:],
                            in0=drl[:, j : j + 1].to_broadcast([P, P]),
                            in1=iota_b[:],
                            op=ALU.is_equal,
                        )
                        Aw = ap.tile([P, P], dt.bfloat16, tag="Aw")
                        nc.scalar.activation(
                            Aw[:], A[:], AF.Copy, scale=ewt[:, j : j + 1]
                        )
                        nc.tensor.matmul(
                            out=py[:],
                            lhsT=xg_bufs[bkt][:, cb, :],
                            rhs=Aw[:],
                            start=(j == 0),
                            stop=(j == CT - 1),
                        )
                    consume(t, py)

            with (
                tc.tile_pool(name="ttp", bufs=3) as ttp,
                tc.tile_pool(name="tpp", bufs=2, space="PSUM") as tpp,
                tc.tile_pool(name="gp", bufs=2) as gp,
                tc.tile_pool(name="ap", bufs=3) as ap,
                tc.tile_pool(name="ypp", bufs=2, space="PSUM") as ypp,
                tc.tile_pool(name="wp", bufs=2) as wp,
                tc.tile_pool(name="wpp", bufs=2, space="PSUM") as wpp,
                tc.tile_pool(name="stg", bufs=2) as stgp,
            ):
                tc.strict_bb_all_engine_barrier()
                for l in range(NGL):
                    # table A from x_temp (f32)
                    build_table(x_temp, True, xsrc_a, xfull_a, (ttp, tpp))

                    # prop1 -> t1 = -dinv * agg (bf16)
                    def consume_t1(t, py):
                        sl = slice(t * P, (t + 1) * P)
                        nc.vector.tensor_tensor(
                            out=t1[:, sl], in0=py[:], in1=drep1[:, sl], op=ALU.mult
                        )

                    prop_pass(xfull_a, consume_t1, (gp, ap, ypp))

                    if debug_outputs and l == 0:
                        nc.sync.dma_start(out=dbg_ew1[None, :], in_=ew1_d[None, :])
                        nc.sync.dma_start(out=dbg_dinv[None, :], in_=dinv_d[None, :])
                        nc.sync.dma_start(out=dbg_xfull[:, :], in_=xfull_a[:, :])
                        dbg_t1_sb = ttp.tile(
                            [F, DTP], dt.bfloat16, tag="dbg_t1_sb", bufs=1
                        )
                        nc.vector.tensor_copy(dbg_t1_sb[:], t1[:])
                        nc.sync.dma_start(out=dbg_t1[:, :], in_=dbg_t1_sb[:])
                        nc.sync.dma_start(out=dbg_drep[:, :], in_=drep1[:])
                        tc.strict_bb_all_engine_barrier()

                    # table B from t1 (bf16)
                    build_table(t1, False, xsrc_b, xfull_b, (ttp, tpp))

                    # prop2 -> s2stage = prop(t1) (bf16)
                    def consume_s2(t, py):
                        sl = slice(t * P, (t + 1) * P)
                        nc.vector.tensor_tensor(
                            out=s2stage[:, sl], in0=py[:], in1=drep1[:, sl], op=ALU.mult
                        )

                    prop_pass(xfull_b, consume_s2, (gp, ap, ypp))
                    tc.strict_bb_all_engine_barrier()

                    # W phase: blocks of 512 nodes
                    c0 = 0
                    while c0 < DTP:
                        nb = min(4 * P, DTP - c0)
                        # t2 = 2*prop(t1) - x_temp
                        t2b = wp.tile([P, 4 * P], dt.bfloat16, tag="t2b")
                        nc.vector.tensor_scalar(
                            out=t2b[:, :nb],
                            in0=s2stage[:, c0 : c0 + nb],
                            scalar1=2.0,
                            scalar2=None,
                            op0=ALU.mult,
                        )
                        nc.vector.tensor_tensor(
                            out=t2b[:, :nb],
                            in0=t2b[:, :nb],
                            in1=x_temp[:, c0 : c0 + nb],
                            op=ALU.subtract,
                        )
                        xb = wp.tile([P, 4 * P], dt.bfloat16, tag="xb")
                        nc.vector.tensor_copy(xb[:, :nb], x_temp[:, c0 : c0 + nb])
                        po = wpp.tile([P, 4 * P], dt.float32, tag="po", space="PSUM")
                        nc.tensor.matmul(
                            out=po[:, :nb],
                            lhsT=W0_sb[:, l * F : (l + 1) * F],
                            rhs=xb[:, :nb],
                            start=True,
                            stop=False,
                        )
                        nc.tensor.matmul(
                            out=po[:, :nb],
                            lhsT=Wb_sb[:, (l * 2) * F : (l * 2 + 1) * F],
                            rhs=t1[:, c0 : c0 + nb],
                            start=False,
                            stop=False,
                        )
                        nc.tensor.matmul(
                            out=po[:, :nb],
                            lhsT=Wb_sb[:, (l * 2 + 1) * F : (l * 2 + 2) * F],
                            rhs=t2b[:, :nb],
                            start=False,
                            stop=True,
                        )
                        if l == 0:
                            nc.scalar.activation(
                                x_temp[:, c0 : c0 + nb],
                                po[:, :nb],
                                AF.Relu,
                                bias=bT_sb[:, l : l + 1],
                            )
                        elif l < NGL - 1:
                            xn = wp.tile([P, 4 * P], dt.float32, tag="xn")
                            nc.scalar.activation(
                                xn[:, :nb],
                                po[:, :nb],
                                AF.Relu,
                                bias=bT_sb[:, l : l + 1],
                            )
                            nc.vector.tensor_tensor(
                                out=x_temp[:, c0 : c0 + nb],
                                in0=x_temp[:, c0 : c0 + nb],
                                in1=xn[:, :nb],
                                op=ALU.add,
                            )
                        else:
                            xo = wp.tile([P, 4 * P], dt.float32, tag="xo")
                            nc.scalar.activation(
                                xo[:, :nb],
                                po[:, :nb],
                                AF.Relu,
                                bias=bT_sb[:, l : l + 1],
                            )
                            nc.sync.dma_start(
                                out=outT_d[:, c0 : c0 + nb], in_=xo[:, :nb]
                            )
                        c0 += nb
    if do_compile:
        nc.compile()
    return nc


# ----------------------------------------------------------------------------
# entry point
# ----------------------------------------------------------------------------
_CACHE = {}


def kernel(**inputs):
    n_nodes, fdim = inputs["features"].shape
    n_edges = inputs["edge_index"].shape[1]
    cfg = Cfg(n_nodes, n_edges, 8)
    in_maps, _meta = host_prep(inputs, cfg)

    from concourse.bass_utils import run_bass_kernel_spmd

    key = (cfg.N, cfg.E, cfg.CT, cfg.CT2, cfg.CTB)
    if key not in _CACHE:
        _CACHE[key] = build_nc(cfg)
    nc = _CACHE[key]
    res = run_bass_kernel_spmd(nc, in_maps, core_ids=list(range(cfg.NC)))
    return assemble(res.results, cfg)


if __name__ == "__main__":
    pass

